# revision 30
# baseline (speedup 1.0000x reference)
"""Trainium2 Bass kernel for nn_MultiHeadModulator (8-core SPMD).

Math reformulation (exact): with a single query q = Wq@z_curr+bq,
  - dot scores:  score[l,h] = z[l]·A[:,h] + c[h],   A[:,h] = Wk[hb,:]^T @ q[hb]
  - rel scores fold into a per-(l,h) additive bias known on the host
  - value sum:   sum_l e[l,h]*v[l] = Wv @ (sum_l e[l,h]*z[l]) + (sum_l e[l,h])*bv
so the device only computes, per L-shard:
  score^T = A^T z^T   (PE, fp8 DoubleRow),  e^T = exp(scale*score + c_h) * fac
  U[h,:] += e^T z     (PE, fp8 DoubleRow),  S[h] from exp's accum_out
and the host applies Wv/Wo and the softmax normalization to the tiny [8,512]
all-core sums.  Softmax runs without max-subtraction: scores are O(1) by
construction (validated |score| < 3).

Sharding: z_past split into 8 contiguous shards of 8192 rows, one per core.
The host ships each shard twice (feature-major for scores, row-major for U)
in fp8, pre-packed for DoubleRow access patterns (the dual layout costs 2x
HBM but avoids any on-chip transpose of z; only the tiny e^T [8,512] tiles
get PE-transposed per block).

Measured: ~41.2 us HW exec (8 cores), rel err 5.8e-3 vs the f32 reference.
Roofline: ~23.5 us of per-core HBM traffic (8.4 MB @ 358 GB/s) + ~7.5 us
fixed NEFF preamble + ~3 us tail.

Scheduling notes (hard-won):
  - all bulk loads ride the sync HWDGE ring, dispatched before the compute
    loop in consumption order (zt0, zn0, zt1, ...); a_dr goes on the scalar
    ring in parallel.  Bulk DMAs emitted inside the block loop get
    interleaved AFTER exp instructions on the scalar sequencer and stall.
    gpsimd/SWDGE first-byte is ~10 us - never put early loads there.
  - weight-side DoubleRow LDWEIGHTS requires the pair-dim step to be a
    multiple of 16 elements (hence the [.., 16]-padded weight layouts).
  - nc.vector.tensor_tensor_reduce crashes on HW (fine in CoreSim); S uses
    the exp's accum_out for uncorrected blocks + a DVE reduce for block 0.
  - PSUM budget (8 banks): 4x score + 3x e-transpose + 1x U accumulator.

Optimization attempts that did NOT beat this schedule (perfetto-verified,
kept for the next session):
  - fanning bulk loads across rings: scalar HWDGE sustains only ~35-55GB/s
    and gpsimd SWDGE ~115GB/s (+~11us boot), and BOTH actively steal from
    the sync ring's throughput while streaming - every multi-ring variant
    measured 5-8us SLOWER end-to-end (44.7-50.9us).
  - ~10 big triggers instead of 35 small ones: the sync ring then moves all
    8.4MB by ~25-30us (it sustains ~420GB/s when fed big D2Ds!), but the
    compute chain (ACT exp serial chain + PE ping-pong on the 4-deep sc
    PSUM rotation) cannot consume faster than the trickle anyway: 43.9us.
  - two-deep software pipelining (sc b+1 | T b | U b-1) + dropping the
    exp accum_out (S via idle-DVE tensor_reduce) + sc PSUM runway of 5:
    ties this schedule at 42.5us - PE matmuls do reach the ramped DVFS
    p-state (216ns vs 379ns per 512-col fp8-DR stream), but the end-to-end
    critical path stays DMA-trickle + fixed ~7.5us preamble + ~4us tail.
"""

import numpy as np
import ml_dtypes

import concourse.bass as bass  # noqa: F401  (engine namespaces live on the nc)
import concourse.mybir as mybir
import concourse.tile as tile
from concourse import bacc
from concourse.bass_utils import run_bass_kernel_spmd

HEADS = 8
REL_MAX = 64
DIM = 256
D2 = 512                      # flattened real feature dim
HD = DIM // HEADS             # 32 complex => 64 reals per head block
L_TOTAL = 65536
N_CORES = 8
L_SHARD = L_TOTAL // N_CORES  # 8192
N_BLOCKS = L_SHARD // 512     # 16 blocks of 512 rows
BLK_PER_SUPER = 4             # blocks per DMA (1 MB chunks)
N_SUPER = N_BLOCKS // BLK_PER_SUPER
SCALE = 1.0 / np.sqrt(HD)

FP8 = ml_dtypes.float8_e4m3   # == mybir.dt.float8e4 (trainium E4M3, max 240)
BF16 = ml_dtypes.bfloat16

TRACE = False                 # test.py can flip this for profiling runs
TRACE_KW = {}

_cached = {}


def _build_program(full_fac: bool):
    nc = bacc.Bacc(
        "TRN2", target_bir_lowering=False, debug=False, num_devices=N_CORES
    )
    DR = mybir.MatmulPerfMode.DoubleRow
    f8 = mybir.dt.float8e4

    ZT = nc.dram_tensor(
        "zt", [N_SUPER, 128, BLK_PER_SUPER, 2, 2, 512], f8, kind="ExternalInput"
    )
    ZN = nc.dram_tensor(
        "zn", [N_SUPER, 128, BLK_PER_SUPER, 2, 2, 512], f8, kind="ExternalInput"
    )
    AT = nc.dram_tensor("a_dr", [128, 2, 2, 16], f8, kind="ExternalInput")
    FAC = nc.dram_tensor(
        "fac", [8, L_SHARD if full_fac else 512], mybir.dt.bfloat16,
        kind="ExternalInput",
    )
    CB = nc.dram_tensor("cb", [8, 1], mybir.dt.float32, kind="ExternalInput")
    IDENT = nc.dram_tensor("ident", [8, 8], mybir.dt.bfloat16, kind="ExternalInput")
    # single output: cols 0..511 = U, cols 512..527 = per-block S partials
    OUT = nc.dram_tensor("out", [8, 528], mybir.dt.float32, kind="ExternalOutput")

    with tile.TileContext(nc) as tc:
        with (
            tc.tile_pool(name="zt", bufs=N_SUPER) as zt_pool,
            tc.tile_pool(name="zn", bufs=N_SUPER) as zn_pool,
            tc.tile_pool(name="consts", bufs=1) as const_pool,
            tc.tile_pool(name="et", bufs=6) as et_pool,
            tc.tile_pool(name="e8", bufs=6) as e8_pool,
            tc.tile_pool(name="outs", bufs=1) as out_pool,
            tc.tile_pool(name="ps_sc", bufs=5, space="PSUM") as sc_pool,
            tc.tile_pool(name="ps_etp", bufs=2, space="PSUM") as etp_pool,
            tc.tile_pool(name="ps_acc", bufs=1, space="PSUM") as acc_pool,
        ):
            # a_dr rides the scalar ring (tiny, lands early in parallel with
            # the sync ring's zt0)
            a_sb = const_pool.tile([128, 2, 2, 16], f8)
            nc.scalar.dma_start(a_sb[:], AT[:])

            zt_tiles = [None] * N_SUPER
            zn_tiles = [None] * N_SUPER

            def load_super(sup):
                zt_s = zt_pool.tile([128, BLK_PER_SUPER, 2, 2, 512], f8, tag="zt_s")
                zn_s = zn_pool.tile([128, BLK_PER_SUPER, 2, 2, 512], f8, tag="zn_s")
                nc.sync.dma_start(zt_s[:], ZT[sup])
                nc.sync.dma_start(zn_s[:], ZN[sup])
                zt_tiles[sup] = zt_s
                zn_tiles[sup] = zn_s

            load_super(0)
            cb_sb = const_pool.tile([8, 1], mybir.dt.float32)
            nc.sync.dma_start(cb_sb[:], CB[:])
            id_sb = const_pool.tile([8, 8], mybir.dt.bfloat16)
            nc.sync.dma_start(id_sb[:], IDENT[:])
            fac_sb = const_pool.tile(
                [8, L_SHARD if full_fac else 512], mybir.dt.bfloat16
            )
            nc.sync.dma_start(fac_sb[:], FAC[:])
            for sup in range(1, N_SUPER):
                load_super(sup)

            u_ps = acc_pool.tile([8, 512], mybir.dt.float32)
            out_sb = out_pool.tile([8, 528], mybir.dt.float32)

            def scores(b):
                # score^T[h, l] for the block's 512 rows, K=512 via 2x DoubleRow
                sup, blk = divmod(b, BLK_PER_SUPER)
                zt_t = zt_tiles[sup][:, blk]
                sc = sc_pool.tile(
                    [8, 512], mybir.dt.float32, tag="sc", name=f"sc_{b}"
                )
                for cpair in range(2):
                    nc.tensor.matmul(
                        sc[:],
                        a_sb[:, cpair, :, 0:8],
                        zt_t[:, cpair],
                        start=(cpair == 0),
                        stop=(cpair == 1),
                        perf_mode=DR,
                    )
                et = et_pool.tile(
                    [8, 512], mybir.dt.bfloat16, tag="et", name=f"et_{b}"
                )
                nc.scalar.activation(
                    et[:],
                    sc[:],
                    mybir.ActivationFunctionType.Exp,
                    bias=cb_sb[:, 0:1],
                    scale=float(SCALE),
                )
                # rel-bias correction factors: only block 0 deviates from 1
                # in the common curr_pos regime (full_fac covers the rest)
                if full_fac or b == 0:
                    etc = et_pool.tile(
                        [8, 512], mybir.dt.bfloat16, tag="etc", name=f"etc_{b}"
                    )
                    nc.vector.tensor_mul(
                        etc[:], et[:], fac_sb[:, 512 * b : 512 * (b + 1)]
                    )
                else:
                    etc = et
                # S on the idle DVE so the ACT chain stays pure exp (no
                # 185ns ACTIVATION_READ_ACCUMULATOR serializing it)
                nc.vector.tensor_reduce(
                    out_sb[:, 512 + b : 513 + b],
                    etc[:],
                    axis=mybir.AxisListType.X,
                    op=mybir.AluOpType.add,
                )
                return etc

            def transposes(b, etc):
                # transpose e^T -> e[l,h] in 4x [8,128] chunks (PE+identity)
                etp = etp_pool.tile(
                    [128, 4, 8], mybir.dt.bfloat16, tag="etp", name=f"etp_{b}"
                )
                for quad in range(4):
                    nc.tensor.transpose(
                        etp[:, quad],
                        etc[:, 128 * quad : 128 * (quad + 1)],
                        id_sb[:],
                    )
                e8 = e8_pool.tile([128, 4, 16], f8, tag="e8", name=f"e8_{b}")
                nc.vector.tensor_copy(e8[:, :, 0:8], etp[:])
                return e8

            def weighted_sum(b, e8, first, last):
                sup, blk = divmod(b, BLK_PER_SUPER)
                zn_t = zn_tiles[sup][:, blk]
                for s in range(2):
                    nc.tensor.matmul(
                        u_ps[:],
                        e8[:, 2 * s : 2 * s + 2, 0:8],
                        zn_t[:, s],
                        start=(first and s == 0),
                        stop=(last and s == 1),
                        perf_mode=DR,
                    )

            # two-deep software pipeline; each PE iteration runs
            #   [scores(b) | U-matmul(b-2) | transposes(b-1)]
            # U before T gives exp(b-1) a full extra matmul of slack, so
            # the PE stream stays dense (keeps the DVFS p-state ramped)
            e8s = {}
            etcs = {}
            for b in range(N_BLOCKS):
                etcs[b] = scores(b)
                if b >= 2:
                    weighted_sum(b - 2, e8s.pop(b - 2), b == 2, False)
                if b >= 1:
                    e8s[b - 1] = transposes(b - 1, etcs.pop(b - 1))
            weighted_sum(N_BLOCKS - 2, e8s.pop(N_BLOCKS - 2), False, False)
            e8s[N_BLOCKS - 1] = transposes(
                N_BLOCKS - 1, etcs.pop(N_BLOCKS - 1)
            )
            weighted_sum(N_BLOCKS - 1, e8s.pop(N_BLOCKS - 1), False, True)

            # final U copy on ACT (idle at kernel end, sits closest to PSUM)
            nc.scalar.copy(out_sb[:, 0:512], u_ps[:])
            nc.sync.dma_start(OUT[:], out_sb[:])

    nc.compile()
    return nc


def _get_program(full_fac: bool):
    if full_fac not in _cached:
        _cached[full_fac] = _build_program(full_fac)
    return _cached[full_fac]


def kernel(curr_pos, z_curr, z_past, Wq, bq, Wk, bk, Wv, bv, Wo, bo, rel_bias):
    curr_pos = int(np.asarray(curr_pos))
    z_curr = np.asarray(z_curr, dtype=np.float32)
    z_past = np.asarray(z_past, dtype=np.float32)
    Wq = np.asarray(Wq, dtype=np.float32)
    bq = np.asarray(bq, dtype=np.float32)
    Wk = np.asarray(Wk, dtype=np.float32)
    bk = np.asarray(bk, dtype=np.float32)
    Wv = np.asarray(Wv, dtype=np.float32)
    bv = np.asarray(bv, dtype=np.float32)
    Wo = np.asarray(Wo, dtype=np.float32)
    bo = np.asarray(bo, dtype=np.float32)
    rel_bias = np.asarray(rel_bias, dtype=np.float32)

    # ---- host-side O(D^2) prep (f64) ----
    q = z_curr.reshape(-1).astype(np.float64) @ Wq.T.astype(np.float64) + bq
    A = np.zeros((D2, HEADS), np.float64)
    c = np.zeros(HEADS, np.float64)
    for h in range(HEADS):
        sl = slice(h * 2 * HD, (h + 1) * 2 * HD)
        A[:, h] = Wk[sl, :].T.astype(np.float64) @ q[sl]
        c[h] = bk[sl].astype(np.float64) @ q[sl]
    relflat = rel_bias.reshape(2 * REL_MAX + 1, D2).astype(np.float64)
    rb = np.stack(
        [
            relflat[:, h * 2 * HD : (h + 1) * 2 * HD] @ q[h * 2 * HD : (h + 1) * 2 * HD]
            for h in range(HEADS)
        ],
        axis=1,
    )  # [129, 8]
    idx = np.clip(
        curr_pos - L_TOTAL + np.arange(L_TOTAL) + REL_MAX, 0, 2 * REL_MAX
    ).astype(np.int64)

    z8 = np.clip(z_past.reshape(L_TOTAL, D2), -240.0, 240.0).astype(FP8)
    A8 = np.clip(A, -240.0, 240.0).astype(np.float32).astype(FP8)
    a_dr = np.zeros((128, 2, 2, 16), FP8)
    a_dr[:, :, :, 0:8] = A8.reshape(2, 2, 128, HEADS).transpose(2, 0, 1, 3)

    ident = np.eye(8, dtype=BF16)

    in_maps = []
    facs = []
    for core in range(N_CORES):
        zc = z8[core * L_SHARD : (core + 1) * L_SHARD]
        # zt[sup, p, blk, cpair, d, l] = zc[512*(2*sup+blk) + l, 256*cpair + 128d + p]
        zt = np.ascontiguousarray(
            zc.reshape(N_SUPER, BLK_PER_SUPER, 512, 2, 2, 128).transpose(
                0, 5, 1, 3, 4, 2
            )
        )
        # zn[sup, p, blk, s, d, f] = zc[512*(2*sup+blk) + 256s + 128d + p, f]
        zn = np.ascontiguousarray(
            zc.reshape(N_SUPER, BLK_PER_SUPER, 2, 2, 128, D2).transpose(
                0, 4, 1, 2, 3, 5
            )
        )
        idx_c = idx[core * L_SHARD : (core + 1) * L_SHARD]
        base = int(np.bincount(idx_c, minlength=2 * REL_MAX + 1).argmax())
        cb = ((c + rb[base]) * SCALE).astype(np.float32).reshape(HEADS, 1)
        fac = np.ascontiguousarray(
            np.exp((rb[idx_c] - rb[base]) * SCALE).T.astype(BF16)
        )
        facs.append(fac)
        in_maps.append(
            {
                "zt": zt,
                "zn": zn,
                "a_dr": a_dr,
                "fac": fac,
                "cb": cb,
                "ident": ident,
            }
        )

    # fast path: correction factors are 1.0 outside block 0 on every core
    full_fac = any(
        not np.all(f[:, 512:] == np.asarray(1.0, BF16)) for f in facs
    )
    if not full_fac:
        for m in in_maps:
            m["fac"] = np.ascontiguousarray(m["fac"][:, 0:512])
    nc = _get_program(full_fac)
    res = run_bass_kernel_spmd(
        nc, in_maps, list(range(N_CORES)), trace=TRACE, **TRACE_KW
    )
    if TRACE:
        kernel.last_result = res

    U = np.zeros((HEADS, D2), np.float64)
    S = np.zeros(HEADS, np.float64)
    for r in res.results:
        o = np.asarray(r["out"], dtype=np.float64)
        U += o[:, 0:512]
        S += o[:, 512:528].sum(axis=1)

    hvec = np.zeros(D2, np.float64)
    for h in range(HEADS):
        sl = slice(h * 2 * HD, (h + 1) * 2 * HD)
        hvec[sl] = Wv[sl, :].astype(np.float64) @ (U[h] / S[h]) + bv[sl]
    out = hvec @ Wo.T.astype(np.float64) + bo
    return out.reshape(DIM, 2).astype(np.float32)


# revision 32
# speedup vs baseline: 1.1366x; 1.1366x over previous
"""Trainium2 Bass kernel for nn_MultiHeadModulator (8-core SPMD).

Math reformulation (exact): with a single query q = Wq@z_curr+bq,
  - dot scores:  score[l,h] = z[l]·A[:,h] + c[h],   A[:,h] = Wk[hb,:]^T @ q[hb]
  - rel scores fold into a per-(l,h) additive bias known on the host
  - value sum:   sum_l e[l,h]*v[l] = Wv @ (sum_l e[l,h]*z[l]) + (sum_l e[l,h])*bv
so the device only computes, per L-shard:
  score^T = A^T z^T   (PE, fp8 DoubleRow),  e^T = exp(scale*score + c_h) * fac
  U[h,:] += e^T z     (PE, fp8 DoubleRow),  S[h] from exp's accum_out
and the host applies Wv/Wo and the softmax normalization to the tiny [8,512]
all-core sums.  Softmax runs without max-subtraction: scores are O(1) by
construction (validated |score| < 3).

Sharding: z_past split into 8 contiguous shards of 8192 rows, one per core.
The host ships each shard twice (feature-major for scores, row-major for U)
in fp8, pre-packed for DoubleRow access patterns (the dual layout costs 2x
HBM but avoids any on-chip transpose of z; only the tiny e^T [8,512] tiles
get PE-transposed per block).

Measured: ~41.2 us HW exec (8 cores), rel err 5.8e-3 vs the f32 reference.
Roofline: ~23.5 us of per-core HBM traffic (8.4 MB @ 358 GB/s) + ~7.5 us
fixed NEFF preamble + ~3 us tail.

Scheduling notes (hard-won):
  - all bulk loads ride the sync HWDGE ring, dispatched before the compute
    loop in consumption order (zt0, zn0, zt1, ...); a_dr goes on the scalar
    ring in parallel.  Bulk DMAs emitted inside the block loop get
    interleaved AFTER exp instructions on the scalar sequencer and stall.
    gpsimd/SWDGE first-byte is ~10 us - never put early loads there.
  - weight-side DoubleRow LDWEIGHTS requires the pair-dim step to be a
    multiple of 16 elements (hence the [.., 16]-padded weight layouts).
  - nc.vector.tensor_tensor_reduce crashes on HW (fine in CoreSim); S uses
    the exp's accum_out for uncorrected blocks + a DVE reduce for block 0.
  - PSUM budget (8 banks): 4x score + 3x e-transpose + 1x U accumulator.

Optimization attempts that did NOT beat this schedule (perfetto-verified,
kept for the next session):
  - fanning bulk loads across rings: scalar HWDGE sustains only ~35-55GB/s
    and gpsimd SWDGE ~115GB/s (+~11us boot), and BOTH actively steal from
    the sync ring's throughput while streaming - every multi-ring variant
    measured 5-8us SLOWER end-to-end (44.7-50.9us).
  - ~10 big triggers instead of 35 small ones: the sync ring then moves all
    8.4MB by ~25-30us (it sustains ~420GB/s when fed big D2Ds!), but the
    compute chain (ACT exp serial chain + PE ping-pong on the 4-deep sc
    PSUM rotation) cannot consume faster than the trickle anyway: 43.9us.
  - two-deep software pipelining (sc b+1 | T b | U b-1) + dropping the
    exp accum_out (S via idle-DVE tensor_reduce) + sc PSUM runway of 5:
    ties this schedule at 42.5us - PE matmuls do reach the ramped DVFS
    p-state (216ns vs 379ns per 512-col fp8-DR stream), but the end-to-end
    critical path stays DMA-trickle + fixed ~7.5us preamble + ~4us tail.
"""

import numpy as np
import ml_dtypes

import concourse.bass as bass  # noqa: F401  (engine namespaces live on the nc)
import concourse.mybir as mybir
import concourse.tile as tile
from concourse import bacc
from concourse.bass_utils import run_bass_kernel_spmd

HEADS = 8
REL_MAX = 64
DIM = 256
D2 = 512                      # flattened real feature dim
HD = DIM // HEADS             # 32 complex => 64 reals per head block
L_TOTAL = 65536
N_CORES = 8
L_SHARD = L_TOTAL // N_CORES  # 8192
N_BLOCKS = L_SHARD // 512     # 16 blocks of 512 rows
BLK_PER_SUPER = 4             # blocks per DMA (1 MB chunks)
N_SUPER = N_BLOCKS // BLK_PER_SUPER
SCALE = 1.0 / np.sqrt(HD)

FP8 = ml_dtypes.float8_e4m3   # == mybir.dt.float8e4 (trainium E4M3, max 240)
BF16 = ml_dtypes.bfloat16

TRACE = False                 # test.py can flip this for profiling runs
TRACE_KW = {}

_cached = {}


def _build_program(full_fac: bool):
    nc = bacc.Bacc(
        "TRN2", target_bir_lowering=False, debug=False, num_devices=N_CORES
    )
    DR = mybir.MatmulPerfMode.DoubleRow
    f8 = mybir.dt.float8e4

    facw = L_SHARD if full_fac else 512
    # block 0 of zt with a_dr packed into cols 512:528 (528 = 33*16 keeps
    # the DoubleRow pair-dim step a multiple of 16)
    ZT0 = nc.dram_tensor("zt0", [128, 2, 2, 528], f8, kind="ExternalInput")
    ZT1 = nc.dram_tensor("zt1", [128, 3, 2, 2, 512], f8, kind="ExternalInput")
    ZN1 = nc.dram_tensor("zn1", [128, 4, 2, 2, 512], f8, kind="ExternalInput")
    ZTS = nc.dram_tensor(
        "zts", [3, 128, 4, 2, 2, 512], f8, kind="ExternalInput"
    )
    ZNS = nc.dram_tensor(
        "zns", [3, 128, 4, 2, 2, 512], f8, kind="ExternalInput"
    )
    # col 0: cb (bf16; per-head-constant bias error cancels in U/S),
    # cols 1:9 identity, cols 9:9+facw rel-bias correction factors
    CST = nc.dram_tensor("cst", [8, 9 + facw], mybir.dt.bfloat16,
                         kind="ExternalInput")
    OUT_U = nc.dram_tensor("out_u", [8, 512], mybir.dt.float32,
                           kind="ExternalOutput")
    OUT_S = nc.dram_tensor("out_s", [8, N_BLOCKS], mybir.dt.float32,
                           kind="ExternalOutput")

    with tile.TileContext(nc) as tc:
        with (
            tc.tile_pool(name="zt", bufs=1) as zt_pool,
            tc.tile_pool(name="zn", bufs=1) as zn_pool,
            tc.tile_pool(name="consts", bufs=1) as const_pool,
            tc.tile_pool(name="et", bufs=8) as et_pool,
            tc.tile_pool(name="e8", bufs=8) as e8_pool,
            tc.tile_pool(name="outs", bufs=1) as out_pool,
            tc.tile_pool(name="ps_sc", bufs=5, space="PSUM") as sc_pool,
            tc.tile_pool(name="ps_etp", bufs=2, space="PSUM") as etp_pool,
            tc.tile_pool(name="ps_acc", bufs=1, space="PSUM") as acc_pool,
        ):
            # ~10 big D2D triggers, all on the sync HWDGE ring (sustains
            # ~420GB/s when fed large requests; all 8.4MB lands by ~25-30us)
            zt0_sb = const_pool.tile([128, 2, 2, 528], f8)
            nc.sync.dma_start(zt0_sb[:], ZT0[:])
            cst_sb = const_pool.tile([8, 9 + facw], mybir.dt.bfloat16)
            nc.sync.dma_start(cst_sb[:], CST[:])
            zt1_sb = zt_pool.tile([128, 3, 2, 2, 512], f8)
            nc.sync.dma_start(zt1_sb[:], ZT1[:])
            zn1_sb = zn_pool.tile([128, 4, 2, 2, 512], f8)
            nc.sync.dma_start(zn1_sb[:], ZN1[:])
            zts_tiles = [None] * 3
            zns_tiles = [None] * 3
            for s in range(3):
                zts_tiles[s] = zt_pool.tile(
                    [128, 4, 2, 2, 512], f8, name=f"zts_{s}"
                )
                nc.sync.dma_start(zts_tiles[s][:], ZTS[s])
                zns_tiles[s] = zn_pool.tile(
                    [128, 4, 2, 2, 512], f8, name=f"zns_{s}"
                )
                nc.sync.dma_start(zns_tiles[s][:], ZNS[s])

            u_ps = acc_pool.tile([8, 512], mybir.dt.float32)
            outs_sb = out_pool.tile([8, N_BLOCKS], mybir.dt.float32)
            u_sb = out_pool.tile([8, 512], mybir.dt.float32)

            def zt_view(b):
                if b == 0:
                    return zt0_sb[:, :, :, 0:512]
                if b < 4:
                    return zt1_sb[:, b - 1]
                return zts_tiles[(b - 4) // 4][:, (b - 4) % 4]

            def zn_view(b):
                if b < 4:
                    return zn1_sb[:, b]
                return zns_tiles[(b - 4) // 4][:, (b - 4) % 4]

            def scores(b):
                # score^T[h, l] for the block's 512 rows, K=512 via 2x DoubleRow
                zt_t = zt_view(b)
                sc = sc_pool.tile(
                    [8, 512], mybir.dt.float32, tag="sc", name=f"sc_{b}"
                )
                for cpair in range(2):
                    nc.tensor.matmul(
                        sc[:],
                        zt0_sb[:, cpair, :, 512:520],
                        zt_t[:, cpair] if b else zt0_sb[:, cpair, :, 0:512],
                        start=(cpair == 0),
                        stop=(cpair == 1),
                        perf_mode=DR,
                    )
                et = et_pool.tile(
                    [8, 512], mybir.dt.bfloat16, tag="et", name=f"et_{b}"
                )
                nc.scalar.activation(
                    et[:],
                    sc[:],
                    mybir.ActivationFunctionType.Exp,
                    bias=cst_sb[:, 0:1],
                    scale=float(SCALE),
                )
                # rel-bias correction factors: only block 0 deviates from 1
                # in the common curr_pos regime (full_fac covers the rest)
                if full_fac or b == 0:
                    etc = et_pool.tile(
                        [8, 512], mybir.dt.bfloat16, tag="etc", name=f"etc_{b}"
                    )
                    nc.vector.tensor_mul(
                        etc[:], et[:], cst_sb[:, 9 + 512 * b : 9 + 512 * (b + 1)]
                    )
                else:
                    etc = et
                # S on the idle DVE so the ACT chain stays pure exp (no
                # 185ns ACTIVATION_READ_ACCUMULATOR serializing it)
                nc.vector.tensor_reduce(
                    outs_sb[:, b : b + 1],
                    etc[:],
                    axis=mybir.AxisListType.X,
                    op=mybir.AluOpType.add,
                )
                return etc

            def transposes(b, etc):
                # transpose e^T -> e[l,h] in 4x [8,128] chunks (PE+identity)
                etp = etp_pool.tile(
                    [128, 4, 8], mybir.dt.bfloat16, tag="etp", name=f"etp_{b}"
                )
                for quad in range(4):
                    nc.tensor.transpose(
                        etp[:, quad],
                        etc[:, 128 * quad : 128 * (quad + 1)],
                        cst_sb[:, 1:9],
                    )
                e8 = e8_pool.tile([128, 4, 16], f8, tag="e8", name=f"e8_{b}")
                nc.vector.tensor_copy(e8[:, :, 0:8], etp[:])
                return e8

            def weighted_sum(b, e8, first, last):
                zn_t = zn_view(b)
                for s in range(2):
                    nc.tensor.matmul(
                        u_ps[:],
                        e8[:, 2 * s : 2 * s + 2, 0:8],
                        zn_t[:, s],
                        start=(first and s == 0),
                        stop=(last and s == 1),
                        perf_mode=DR,
                    )

            # two-deep software pipeline; each PE iteration runs
            #   [scores(b) | transposes(b-1) | U-matmul(b-2)]
            # so exp(b-1) hides under scores(b) and the DVE e8-cast of a
            # block never blocks the PE stream (keeps the DVFS p-state up)
            e8s = {}
            etcs = {}
            for b in range(N_BLOCKS):
                etcs[b] = scores(b)
                if b >= 1:
                    e8s[b - 1] = transposes(b - 1, etcs.pop(b - 1))
                if b >= 2:
                    weighted_sum(b - 2, e8s.pop(b - 2), b == 2, False)
            e8s[N_BLOCKS - 1] = transposes(
                N_BLOCKS - 1, etcs.pop(N_BLOCKS - 1)
            )
            weighted_sum(N_BLOCKS - 2, e8s.pop(N_BLOCKS - 2), False, False)
            weighted_sum(N_BLOCKS - 1, e8s.pop(N_BLOCKS - 1), False, True)

            # S partials ride the idle sync ring; ACT (closest to PSUM,
            # free after the last exp) copies U, sync sends it
            nc.sync.dma_start(OUT_S[:], outs_sb[:])
            nc.scalar.copy(u_sb[:], u_ps[:])
            nc.sync.dma_start(OUT_U[:], u_sb[:])

    nc.compile()
    return nc


def _get_program(full_fac: bool):
    if full_fac not in _cached:
        _cached[full_fac] = _build_program(full_fac)
    return _cached[full_fac]


def kernel(curr_pos, z_curr, z_past, Wq, bq, Wk, bk, Wv, bv, Wo, bo, rel_bias):
    curr_pos = int(np.asarray(curr_pos))
    z_curr = np.asarray(z_curr, dtype=np.float32)
    z_past = np.asarray(z_past, dtype=np.float32)
    Wq = np.asarray(Wq, dtype=np.float32)
    bq = np.asarray(bq, dtype=np.float32)
    Wk = np.asarray(Wk, dtype=np.float32)
    bk = np.asarray(bk, dtype=np.float32)
    Wv = np.asarray(Wv, dtype=np.float32)
    bv = np.asarray(bv, dtype=np.float32)
    Wo = np.asarray(Wo, dtype=np.float32)
    bo = np.asarray(bo, dtype=np.float32)
    rel_bias = np.asarray(rel_bias, dtype=np.float32)

    # ---- host-side O(D^2) prep (f64) ----
    q = z_curr.reshape(-1).astype(np.float64) @ Wq.T.astype(np.float64) + bq
    A = np.zeros((D2, HEADS), np.float64)
    c = np.zeros(HEADS, np.float64)
    for h in range(HEADS):
        sl = slice(h * 2 * HD, (h + 1) * 2 * HD)
        A[:, h] = Wk[sl, :].T.astype(np.float64) @ q[sl]
        c[h] = bk[sl].astype(np.float64) @ q[sl]
    relflat = rel_bias.reshape(2 * REL_MAX + 1, D2).astype(np.float64)
    rb = np.stack(
        [
            relflat[:, h * 2 * HD : (h + 1) * 2 * HD] @ q[h * 2 * HD : (h + 1) * 2 * HD]
            for h in range(HEADS)
        ],
        axis=1,
    )  # [129, 8]
    idx = np.clip(
        curr_pos - L_TOTAL + np.arange(L_TOTAL) + REL_MAX, 0, 2 * REL_MAX
    ).astype(np.int64)

    z8 = np.clip(z_past.reshape(L_TOTAL, D2), -240.0, 240.0).astype(FP8)
    A8 = np.clip(A, -240.0, 240.0).astype(np.float32).astype(FP8)
    a_dr = np.zeros((128, 2, 2, 16), FP8)
    a_dr[:, :, :, 0:8] = A8.reshape(2, 2, 128, HEADS).transpose(2, 0, 1, 3)

    in_maps = []
    facs = []
    for core in range(N_CORES):
        zc = z8[core * L_SHARD : (core + 1) * L_SHARD]
        # zt_b[p, cpair, d, l] = zc[512*b + l, 256*cpair + 128*d + p]
        zt_all = np.ascontiguousarray(
            zc.reshape(N_BLOCKS, 512, 2, 2, 128).transpose(0, 4, 2, 3, 1)
        )
        # zn_b[p, s, d, f] = zc[512*b + 256*s + 128*d + p, f]
        zn_all = np.ascontiguousarray(
            zc.reshape(N_BLOCKS, 2, 2, 128, 512).transpose(0, 3, 1, 2, 4)
        )

        def pk(blob, lo, hi):  # [nblk,128,2,2,512] -> [128,nblk,2,2,512]
            return np.ascontiguousarray(blob[lo:hi].transpose(1, 0, 2, 3, 4))

        def pks(blob, lo, hi):  # -> [nsup,128,4,2,2,512]
            n = (hi - lo) // 4
            return np.ascontiguousarray(
                blob[lo:hi].reshape(n, 4, 128, 2, 2, 512).transpose(
                    0, 2, 1, 3, 4, 5
                )
            )

        idx_c = idx[core * L_SHARD : (core + 1) * L_SHARD]
        base = int(np.bincount(idx_c, minlength=2 * REL_MAX + 1).argmax())
        cb = ((c + rb[base]) * SCALE).astype(np.float32).reshape(HEADS, 1)
        fac = np.ascontiguousarray(
            np.exp((rb[idx_c] - rb[base]) * SCALE).T.astype(BF16)
        )
        facs.append(fac)
        in_maps.append(
            {
                "zt0": np.concatenate([zt_all[0], a_dr], axis=3),
                "zt1": pk(zt_all, 1, 4),
                "zn1": pk(zn_all, 0, 4),
                "zts": pks(zt_all, 4, 16),
                "zns": pks(zn_all, 4, 16),
                "cb": cb,
            }
        )

    # fast path: correction factors are 1.0 outside block 0 on every core
    full_fac = any(
        not np.all(f[:, 512:] == np.asarray(1.0, BF16)) for f in facs
    )
    facw = L_SHARD if full_fac else 512
    for core, m in enumerate(in_maps):
        cst = np.zeros((8, 9 + facw), BF16)
        cst[:, 0:1] = m.pop("cb").astype(BF16)
        cst[:, 1:9] = np.eye(8, dtype=BF16)
        cst[:, 9:] = facs[core][:, 0:facw]
        m["cst"] = cst
    nc = _get_program(full_fac)
    res = run_bass_kernel_spmd(
        nc, in_maps, list(range(N_CORES)), trace=TRACE, **TRACE_KW
    )
    if TRACE:
        kernel.last_result = res

    U = np.zeros((HEADS, D2), np.float64)
    S = np.zeros(HEADS, np.float64)
    for r in res.results:
        U += np.asarray(r["out_u"], dtype=np.float64)
        S += np.asarray(r["out_s"], dtype=np.float64).sum(axis=1)

    hvec = np.zeros(D2, np.float64)
    for h in range(HEADS):
        sl = slice(h * 2 * HD, (h + 1) * 2 * HD)
        hvec[sl] = Wv[sl, :].astype(np.float64) @ (U[h] / S[h]) + bv[sl]
    out = hvec @ Wo.T.astype(np.float64) + bo
    return out.reshape(DIM, 2).astype(np.float32)
